# revision 1
# baseline (speedup 1.0000x reference)
"""MeanShift retrieval-KNN loss kernel for 8 Trainium2 NeuronCores — v2.

Reference computation (B=4096, K=32768, DIM=512, TOPK=5):
    query  = l2norm(query_raw); target = l2norm(target_raw)
    qbank  = l2norm(queue); qbank[0:B] = target
    dist_t = 2 - 2 * target @ qbank.T ; dist_q = 2 - 2 * query @ qbank.T
    idx    = top5 smallest dist_t per row
    loss   = mean_b( sum_j dist_q[b, idx[b,j]] / 5 )

Sharding: queue K axis split across 8 cores (4096 rows each); core 0's
shard is target_raw (the reference overwrites bank rows 0:B, and raw
queue rows 0:B are never read).

v2 design vs v1 (bf16 matmuls, ACT y-evac + DVE stt combine + sbuf max8):
  * matmuls in fp8e4 DoubleRow (contraction 256/instr -> 2x PE rate).
  * packed value v = round(2048*sim_t) + sqrt2*sim_q is built INSIDE one
    PSUM accumulation group per 512-chunk:
      ph1 DR (TSC*t @ BSC*bank = 2048*sim_t)
      round: ACT in-place Copy(+MAGIC) then PE ones-matmul(-MAGIC), or
             DVE tensor_scalar (x+MAGIC)-MAGIC   [split is tunable]
      ph2 DR (QSC*q @ BSC*bank = FRS*sim_q) accumulated on top
    -> no separate y array, no DVE combine pass, no v in SBUF.
  * top-8 via DVE max8 straight from each [128,1024] psum pair; 32
    candidates per row per core; host merges 8*32 and decodes
    sim_q = (v - round(v))/FRS.
  * preproc transposes via XBAR dma_start_transpose (bf16, SBUF->SBUF,
    no PE/PSUM involvement); fp8 casts via GPSIMD SWDGE dma-cast.
"""

import numpy as np

B, K, DIM, TOPK = 4096, 32768, 512, 5
NCORES = 8
KSH = K // NCORES  # 4096 bank rows per core

P = 128
CH = 512                      # psum-bank chunk width
MAGIC = float(3 * (2 ** 22))  # fp32 add of +MAGIC snaps to integer grid
TSC = 896.0                   # target scale (fp8_e4m3 max normal = 240)
BSC = 2048.0 / TSC            # bank scale; TSC*BSC = 2048
QSC = 0.5                     # query scale; |BSC*QSC*simq| < 0.5
FRS = BSC * QSC               # packed fraction = FRS * sim_q
NCAND = 32                    # candidates per row per core (4 pairs x 8)

# fraction of chunk-pairs whose round runs ACT(+M)+PE(-M); rest use a
# DVE tensor_scalar round. Tuned from engine-busy traces.
ACT_ROUND_NUM, ACT_ROUND_DEN = 3, 8

_CACHE = {}


def build_nc(b=B, ksh=KSH, dim=DIM, num_devices=NCORES):
    from contextlib import ExitStack

    import concourse.tile as tile
    from concourse import bacc, mybir
    from concourse.masks import make_identity

    f32 = mybir.dt.float32
    bf16 = mybir.dt.bfloat16
    fp8 = mybir.dt.float8e4
    Alu = mybir.AluOpType
    Act = mybir.ActivationFunctionType
    DR = mybir.MatmulPerfMode.DoubleRow

    DCH = dim // P          # 4 transpose chunks / 2 DR slice-pairs
    NB = b // P             # 32 batch tiles
    NPR = ksh // (2 * CH)   # 4 chunk-pairs per batch tile
    NS = ksh // P           # 32 bank row-tiles

    nc = bacc.Bacc(
        "TRN2", target_bir_lowering=False, debug=False, num_devices=num_devices
    )
    q_d = nc.dram_tensor("query_raw", [b, dim], f32, kind="ExternalInput").ap()
    t_d = nc.dram_tensor("target_raw", [b, dim], f32, kind="ExternalInput").ap()
    s_d = nc.dram_tensor("qshard", [ksh, dim], f32, kind="ExternalInput").ap()
    o_d = nc.dram_tensor("out", [b, NCAND], f32, kind="ExternalOutput").ap()

    with tile.TileContext(nc) as tc, ExitStack() as ctx:
        singles = ctx.enter_context(tc.tile_pool(name="singles", bufs=1))
        ld = ctx.enter_context(tc.tile_pool(name="ld", bufs=12))
        nrm = ctx.enter_context(tc.tile_pool(name="nrm", bufs=10))
        small = ctx.enter_context(tc.tile_pool(name="small", bufs=8))
        psum = ctx.enter_context(tc.tile_pool(name="psum", bufs=3, space="PSUM"))
        pstp = ctx.enter_context(tc.tile_pool(name="pstp", bufs=2, space="PSUM"))
        toppool = ctx.enter_context(tc.tile_pool(name="top", bufs=14))

        identb = singles.tile([P, P], bf16)
        make_identity(nc, identb)
        onesc = singles.tile([1, P], bf16)
        nc.gpsimd.memset(onesc, 1.0)
        mrow_n = singles.tile([1, CH], bf16)
        nc.gpsimd.memset(mrow_n, -MAGIC)

        # Resident normalized+scaled+transposed fp8 operands, DIM on parts.
        qbT = singles.tile([P, DCH, ksh], fp8)  # bank^T * BSC
        tT = singles.tile([P, DCH, b], fp8)     # target^T * TSC
        qT = singles.tile([P, DCH, b], fp8)     # query^T

        def preproc(x_dram, dest, it, pfx, scale, sq_eng='act', cast_eng='act',
                    f8_eng='gpdma', trans='xbar'):
            """Load 128 rows -> l2norm*scale -> bf16 -> whole-tile XBAR
            transpose -> fp8 cast into the resident slice.

            sq_eng: 'gp'|'dve'|'act' for the square+accum pass (f32)
            cast_eng: 'act'|'dve' for the normalize-cast to bf16
            f8_eng: 'act'|'dve' for the bf16->fp8 cast copy
            """
            raw = ld.tile([P, dim], f32, tag="raw", name=f"{pfx}r{it}")
            nc.sync.dma_start(out=raw, in_=x_dram[it * P:(it + 1) * P, :])
            sq = nrm.tile([P, dim], f32, tag="sq", name=f"{pfx}sq{it}")
            ss = small.tile([P, 1], f32, tag="ss", name=f"{pfx}ss{it}")
            if sq_eng == 'dve':
                nc.vector.scalar_tensor_tensor(
                    out=sq, in0=raw, scalar=1.0, in1=raw,
                    op0=Alu.mult, op1=Alu.mult, accum_out=ss,
                )
            else:
                nc.scalar.activation(sq, raw, Act.Square, accum_out=ss)
            # stdv = sqrt(ss)/scale ; rin = scale/||x||
            stdv = small.tile([P, 1], f32, tag="std", name=f"{pfx}sd{it}")
            nc.scalar.activation(stdv, ss, Act.Sqrt, scale=1.0 / (scale * scale))
            rin = small.tile([P, 1], f32, tag="rin", name=f"{pfx}ri{it}")
            nc.vector.reciprocal(rin, stdv)
            xn = nrm.tile([P, dim], bf16, tag="xn", name=f"{pfx}xn{it}")
            if cast_eng == 'dve':
                nc.vector.tensor_scalar(out=xn, in0=raw, scalar1=rin,
                                        scalar2=None, op0=Alu.mult)
            else:
                nc.scalar.activation(xn, raw, Act.Copy, scale=rin)
            dslc = dest[:, :, it * P:(it + 1) * P]
            if trans == 'pe':
                # startup path: transpose on the (otherwise idle) PE and
                # evac-cast from PSUM -- avoids XBAR queue latency
                pt = pstp.tile([P, DCH, P], bf16, tag="pt",
                               name=f"{pfx}pt{it}")
                for dc in range(DCH):
                    nc.tensor.transpose(pt[:, dc, :],
                                        xn[:, dc * P:(dc + 1) * P], identb)
                if it % 2 == 0:
                    nc.scalar.copy(dslc, pt)
                else:
                    nc.vector.tensor_copy(dslc, pt)
                return
            xt = nrm.tile([P, DCH, P], bf16, tag="xt", name=f"{pfx}xt{it}")
            # whole-tile XBAR transpose [128,512] -> [128,4,128], always on
            # the scalar HWDGE queue: an XBAR waiting for xn in the sync
            # queue would head-of-line block the input loads behind it.
            nc.scalar.dma_start_transpose(xt, xn)
            # cast bf16 -> fp8 into the strided resident slice
            if f8_eng == 'dve':
                nc.vector.tensor_copy(dslc, xt)
            elif f8_eng == 'gpdma':
                nc.gpsimd.dma_start(out=dslc, in_=xt)  # SWDGE dtype cast
            else:
                nc.scalar.copy(dslc, xt)

        def ph1_round(bt, pr, i):
            """Phase-1 DR matmuls + round for one [128,1024] psum pair.

            The round (ACT or DVE) is emitted immediately so it can run
            while the PE continues with the NEXT pair's phase-1; the
            -MAGIC ones-matmuls + phase-2 are emitted one iteration later
            (software pipelining) so the PE never waits on the round.
            """
            bs = slice(bt * P, (bt + 1) * P)
            pv = psum.tile([P, 2 * CH], f32, tag="pv", name=f"pv{bt}_{pr}")
            for c in range(2):
                kc = pr * 2 + c
                ks = slice(kc * CH, (kc + 1) * CH)
                for dr in range(2):
                    nc.tensor.matmul(pv[:, c * CH:(c + 1) * CH],
                                     tT[:, 2 * dr:2 * dr + 2, bs],
                                     qbT[:, 2 * dr:2 * dr + 2, ks],
                                     start=(dr == 0), stop=False, perf_mode=DR)
            act_round = (i % ACT_ROUND_DEN) < ACT_ROUND_NUM
            if act_round:
                nc.scalar.activation(pv, pv, Act.Copy, bias=MAGIC)
            else:
                nc.vector.tensor_scalar(out=pv, in0=pv, scalar1=MAGIC,
                                        scalar2=-MAGIC, op0=Alu.add,
                                        op1=Alu.add)
            return pv, act_round

        def ph2_max(bt, pr, pv, act_round, cand):
            bs = slice(bt * P, (bt + 1) * P)
            if act_round:
                for c in range(2):
                    nc.tensor.matmul(pv[:, c * CH:(c + 1) * CH], onesc, mrow_n,
                                     start=False, stop=False,
                                     skip_group_check=True)
            for c in range(2):
                kc = pr * 2 + c
                ks = slice(kc * CH, (kc + 1) * CH)
                for dr in range(2):
                    nc.tensor.matmul(pv[:, c * CH:(c + 1) * CH],
                                     qT[:, 2 * dr:2 * dr + 2, bs],
                                     qbT[:, 2 * dr:2 * dr + 2, ks],
                                     start=False, stop=(dr == 1), perf_mode=DR)
            nc.vector.max(cand[:, pr * 8:(pr + 1) * 8], pv)

        # ---- software-pipelined main loop over all (bt, pr) pairs ----
        # Startup is slice-major over the first SB batch tiles: as soon as
        # bank slice 0 is resident, the PE has SB pairs of work while the
        # remaining slices and t/q tiles stream in. After that, bt-major.
        PF = 3
        TPP = NS // NPR  # bank row-tiles per pair-slice (8)
        SB = min(12, NB)

        def bank_tile(j, trans='xbar'):
            preproc(s_d, qbT, j, "s", BSC,
                    sq_eng=('dve' if j % 2 else 'act'),
                    cast_eng=('dve' if j % 2 else 'act'), trans=trans)

        def tq_tile(bt, trans='xbar'):
            preproc(t_d, tT, bt, "t", TSC, trans=trans)
            preproc(q_d, qT, bt, "q", QSC, sq_eng='dve', cast_eng='dve',
                    trans=trans)

        cands = {}
        items = [(bt, pr) for pr in range(NPR) for bt in range(SB)] + \
                [(bt, pr) for bt in range(SB, NB) for pr in range(NPR)]
        pend = None  # (bt, pr, pv, act_round)
        for i, (bt, pr) in enumerate(items):
            startup = i < SB * NPR
            if i == 0:
                tq_tile(0, trans='pe')
                tq_tile(1, trans='pe')
                for j in range(TPP):
                    bank_tile(j, trans='pe')
            if startup:
                if pr == 0 and bt + 2 < SB:
                    tq_tile(bt + 2, trans=('pe' if bt < 2 else 'xbar'))
                if pr + 1 < NPR and bt < TPP:
                    bank_tile((pr + 1) * TPP + bt, trans='pe')
                if pr == NPR - 1 and bt % 4 == 0 and SB + bt // 4 < SB + PF:
                    tq_tile(SB + bt // 4)
            elif pr == 0 and PF <= bt < NB - PF:
                tq_tile(bt + PF)
            if pr == 0 or bt not in cands:
                if bt not in cands:
                    cands[bt] = toppool.tile([P, NCAND], f32, tag="cand",
                                             name=f"cand{bt}")
            pv, ar = ph1_round(bt, pr, i)
            if pend is not None:
                pbt, ppr, ppv, par = pend
                ph2_max(pbt, ppr, ppv, par, cands[pbt])
                if ppr == NPR - 1:
                    nc.gpsimd.dma_start(
                        out=o_d[pbt * P:(pbt + 1) * P, :], in_=cands[pbt])
            pend = (bt, pr, pv, ar)
        pbt, ppr, ppv, par = pend
        ph2_max(pbt, ppr, ppv, par, cands[pbt])
        nc.gpsimd.dma_start(out=o_d[pbt * P:(pbt + 1) * P, :], in_=cands[pbt])

    nc.compile()
    return nc


def _get_nc():
    key = (B, KSH, DIM, NCORES)
    if key not in _CACHE:
        _CACHE[key] = build_nc()
    return _CACHE[key]


def merge_host(cand_v, topk=TOPK):
    """cand_v: [ncores, b, NCAND] packed values -> scalar loss."""
    b = cand_v.shape[1]
    allv = np.transpose(cand_v, (1, 0, 2)).reshape(b, -1)
    part = np.partition(allv, allv.shape[1] - topk, axis=1)[:, -topk:]
    p_int = np.round(part)
    sim_q = (part - p_int) / FRS
    dist_q = 2.0 - 2.0 * sim_q
    return np.float32(dist_q.mean())


def run_device(query_raw, target_raw, queue, **spmd_kwargs):
    from concourse.bass_utils import run_bass_kernel_spmd

    q = np.ascontiguousarray(np.asarray(query_raw, dtype=np.float32))
    t = np.ascontiguousarray(np.asarray(target_raw, dtype=np.float32))
    qu = np.ascontiguousarray(np.asarray(queue, dtype=np.float32))

    nc = _get_nc()
    in_maps = []
    for c in range(NCORES):
        shard = t if c == 0 else qu[c * KSH:(c + 1) * KSH]
        in_maps.append(
            {"query_raw": q, "target_raw": t,
             "qshard": np.ascontiguousarray(shard)}
        )
    bres = run_bass_kernel_spmd(nc, in_maps, list(range(NCORES)), **spmd_kwargs)
    cand = np.stack([bres.results[c]["out"] for c in range(NCORES)], axis=0)
    return merge_host(cand), bres


def kernel(query_raw, target_raw, queue):
    loss, _ = run_device(query_raw, target_raw, queue)
    return loss

